# revision 16
# baseline (speedup 1.0000x reference)
"""GCN (3-layer message passing + mean-pool + MLP head) on 8 Trainium2 NeuronCores.

v2: aggregate-then-matmul formulation.  Per layer the table holds
dinv-scaled *features* (not h'), each core dma_gathers its edges' src rows
(4-way SWDGE queue rotation so descriptor generation runs on all four Q7
core pairs) and segment-sums them feature-major with indicator matmuls
(aggT = sum_k G_k^T-as-lhsT x Ind_k); the self-loop is one identity-matmul
per window against the SBUF-resident local table rows; the W-matmul runs
post-aggregation per window with no transposes.  Layer 1 gathers from a
host-supplied 512B-row [x | dinv-slot] table whose dinv byte is patched on
device after a tiny degree AllGather, so the first 12.8MB feature AllGather
disappears.  Windows run in two passes (L-stream then H-stream) so each
table-half AllGather overlaps the opposite half's compute.  Pooling uses
host-built one-hot graph indicators plus a [129,256] AllReduce; the MLP
head is computed redundantly.
"""

import numpy as np
from dataclasses import dataclass, field


# ---------------------------------------------------------------------------
# Config
# ---------------------------------------------------------------------------
@dataclass
class Cfg:
    N: int = 50000          # nodes
    E: int = 600000         # edges
    F: int = 128            # feature dim
    NL: int = 3             # gcn layers
    G: int = 256            # graphs
    H: int = 256            # hidden dim of head
    LD: int = 2             # label dim
    C: int = 8              # cores
    GCHUNK: int = 1024      # edges per dma_gather call
    IB: int = 8             # indicator subchunks built per DVE op

    @property
    def NPC(self):          # nodes per core
        return self.N // self.C

    @property
    def W(self):            # 128-node windows per core
        return (self.NPC + 127) // 128

    @property
    def NPAD(self):         # padded nodes per core
        return self.W * 128

    @property
    def TROWS(self):        # gather-table rows
        return self.C * self.NPAD

    @property
    def HA(self):           # local rows in table half A (window-aligned)
        return 128 * ((self.W + 1) // 2)

    @property
    def HB(self):           # local rows in table half B
        return self.NPAD - self.HA

    @property
    def SPLIT(self):        # low/high gather-stream boundary = half-A rows
        return self.C * self.HA


@dataclass
class Meta:
    """Uniform (core-independent) graph structure + per-core data arrays."""
    m_low: list = field(default_factory=list)    # per-window low subchunk count
    m_high: list = field(default_factory=list)   # per-window high subchunk count
    L_low: int = 0
    L_high: int = 0
    in_maps: list = field(default_factory=list)  # per-core tensor dicts


# ---------------------------------------------------------------------------
# Host-side sharding / layout prep (pure numpy, no model math)
# ---------------------------------------------------------------------------
def _wrap16(arr_i16):
    # slot i -> [i % 16, i // 16]; 16-row wrap replicated to 128 partitions
    # (one copy per GPSIMD Q7 core).
    return np.ascontiguousarray(np.tile(arr_i16.reshape(-1, 16).T, (8, 1)))


def _wrap128(arr):
    # slot i -> [i % 128, i // 128]
    return np.ascontiguousarray(arr.reshape(-1, 128).T)


def host_prep(cfg: Cfg, x, Wg, bg, w1, b1, w2, b2, edge_index, batch) -> Meta:
    C, NPC, W, NPAD = cfg.C, cfg.NPC, cfg.W, cfg.NPAD
    src = np.asarray(edge_index[0], dtype=np.int64)
    dst = np.asarray(edge_index[1], dtype=np.int64)
    batch = np.asarray(batch, dtype=np.int64)
    x = np.asarray(x, dtype=np.float32)

    # table row of a global node id: rows [0, C*HA) hold every core's first
    # HA local rows, rows [C*HA, TROWS) the remaining HB.
    HA, HB = cfg.HA, cfg.HB
    nid = np.arange(cfg.N, dtype=np.int64)
    nc_, nl = nid // NPC, nid % NPC
    trow_of = np.where(nl < HA, nc_ * HA + nl, C * HA + nc_ * HB + (nl - HA))
    trow = trow_of[src]

    # layer-1 table: row trow(n) = [x[n] bf16 | slot for dinv | zeros], 512B
    import ml_dtypes
    x512 = np.zeros((cfg.TROWS, 256), dtype=ml_dtypes.bfloat16)
    x512[trow_of, 0:128] = x.astype(ml_dtypes.bfloat16)

    # per (core, window, half) edge lists
    per_core = []
    for c in range(C):
        m = (dst // NPC) == c
        s_c, d_c, t_c = src[m], dst[m], trow[m]
        dloc = d_c - c * NPC
        order = np.argsort(dloc, kind="stable")
        s_c, dloc, t_c = s_c[order], dloc[order], t_c[order]
        win = dloc // 128
        drel = dloc - win * 128
        lowm = t_c < cfg.SPLIT
        lists = []
        for w in range(W):
            wm = win == w
            lists.append((
                (t_c[wm & lowm], drel[wm & lowm]),
                (t_c[wm & ~lowm] - cfg.SPLIT, drel[wm & ~lowm]),
            ))
        per_core.append(lists)

    # uniform subchunk counts (max over cores), >=1 low subchunk per window
    m_low = [max(1, max(-(-len(per_core[c][w][0][0]) // 128) for c in range(C)))
             for w in range(W)]
    m_high = [max(-(-len(per_core[c][w][1][0]) // 128) for c in range(C))
              for w in range(W)]
    L_low = 128 * sum(m_low)
    L_high = 128 * sum(m_high)

    meta = Meta(m_low=m_low, m_high=m_high, L_low=L_low, L_high=L_high)

    for c in range(C):
        idx_low = np.zeros(L_low, np.int16)
        drel_low = np.full(L_low, -1.0, np.float32)
        idx_high = np.zeros(max(L_high, 128), np.int16)
        drel_high = np.full(max(L_high, 128), -1.0, np.float32)
        ol = oh = 0
        for w in range(W):
            (tl, dl), (th, dh) = per_core[c][w]
            idx_low[ol:ol + len(tl)] = tl.astype(np.int16)
            drel_low[ol:ol + len(dl)] = dl.astype(np.float32)
            ol += 128 * m_low[w]
            idx_high[oh:oh + len(th)] = th.astype(np.int16)
            drel_high[oh:oh + len(dh)] = dh.astype(np.float32)
            oh += 128 * m_high[w]
        assert ol == L_low and oh == L_high

        # local x shard, node-major bf16 (self-loop matmul lhsT + layer-1
        # local table rows)
        xs = np.zeros((NPAD, cfg.F), ml_dtypes.bfloat16)
        xs[:NPC] = x[c * NPC:(c + 1) * NPC].astype(ml_dtypes.bfloat16)

        # pooling one-hot indicators [node-slot x G] per window, bf16
        pind = np.zeros((NPAD, cfg.G), ml_dtypes.bfloat16)
        bloc = batch[c * NPC:(c + 1) * NPC]
        pind[np.arange(NPC), bloc] = 1.0

        meta.in_maps.append(dict(
            xs=np.ascontiguousarray(xs),
            x512=x512,
            pind=np.ascontiguousarray(pind),
            src_low=_wrap16(idx_low),
            src_high=_wrap16(idx_high),
            drel_low=_wrap128(drel_low.astype(ml_dtypes.bfloat16)),
            drel_high=_wrap128(drel_high.astype(ml_dtypes.bfloat16)),
            Wg=np.asarray(Wg, np.float32),
            bg=np.asarray(bg, np.float32),
            w1=np.asarray(w1, np.float32),
            b1=np.asarray(b1, np.float32).reshape(cfg.H, 1),
            w2=np.asarray(w2, np.float32),
            b2=np.asarray(b2, np.float32).reshape(1, cfg.LD),
        ))
    return meta


# ---------------------------------------------------------------------------
# Device graph
# ---------------------------------------------------------------------------
def build_graph(cfg: Cfg, meta: Meta):
    import concourse.bass as bass
    import concourse.bacc as bacc
    import concourse.mybir as mybir
    import concourse.tile as tile

    F, W, NL, NPAD = cfg.F, cfg.W, cfg.NL, cfg.NPAD
    GR = cfg.G
    f32, bf16, i16 = mybir.dt.float32, mybir.dt.bfloat16, mybir.dt.int16
    AL = mybir.AluOpType
    ACT = mybir.ActivationFunctionType
    L_low, L_high = meta.L_low, meta.L_high
    LH_pad = max(L_high, 128)
    HA, WA = cfg.HA, cfg.HA // 128
    TROWS = cfg.TROWS
    WT = TROWS // 128

    nc = bacc.Bacc("TRN2", target_bir_lowering=False, debug=False,
                   num_devices=cfg.C, num_swdge_queues=4)

    # --- external IO ------------------------------------------------------
    P = {}
    P["xs"] = nc.declare_dram_parameter("xs", [NPAD, F], bf16, isOutput=False)
    P["x512"] = nc.declare_dram_parameter("x512", [TROWS, 256], bf16, isOutput=False)
    P["pind"] = nc.declare_dram_parameter("pind", [NPAD, GR], bf16, isOutput=False)
    P["src_low"] = nc.declare_dram_parameter("src_low", [128, L_low // 16], i16, isOutput=False)
    P["src_high"] = nc.declare_dram_parameter("src_high", [128, LH_pad // 16], i16, isOutput=False)
    P["drel_low"] = nc.declare_dram_parameter("drel_low", [128, L_low // 128], bf16, isOutput=False)
    P["drel_high"] = nc.declare_dram_parameter("drel_high", [128, LH_pad // 128], bf16, isOutput=False)
    P["Wg"] = nc.declare_dram_parameter("Wg", [NL, F, F], f32, isOutput=False)
    P["bg"] = nc.declare_dram_parameter("bg", [NL, F], f32, isOutput=False)
    P["w1"] = nc.declare_dram_parameter("w1", [F, cfg.H], f32, isOutput=False)
    P["b1"] = nc.declare_dram_parameter("b1", [cfg.H, 1], f32, isOutput=False)
    P["w2"] = nc.declare_dram_parameter("w2", [cfg.H, cfg.LD], f32, isOutput=False)
    P["b2"] = nc.declare_dram_parameter("b2", [1, cfg.LD], f32, isOutput=False)
    out_ext = nc.declare_dram_parameter("out", [GR, cfg.LD], f32, isOutput=True)

    # --- internal DRAM ----------------------------------------------------
    tableDs = [nc.dram_tensor(f"tableD{i}", [TROWS, F], bf16,
                              addr_space="Shared") for i in range(2)]
    shardDs = [nc.dram_tensor(f"shardD{i}", [NPAD, F], bf16) for i in range(2)]
    degD = nc.dram_tensor("degD", [NPAD], f32)
    degAllD = nc.dram_tensor("degAllD", [TROWS], f32, addr_space="Shared")
    arInD = nc.dram_tensor("arInD", [129, GR], f32)
    arOutD = nc.dram_tensor("arOutD", [129, GR], f32, addr_space="Shared")

    rg = [list(range(cfg.C))]

    with tile.TileContext(nc) as tc:
        with (
            tc.tile_pool(name="res", bufs=1) as res,      # resident tensors
            tc.tile_pool(name="work", bufs=3) as work,    # per-window temps
            tc.tile_pool(name="indp", bufs=6) as indp,    # indicator batches
            tc.tile_pool(name="gbuf", bufs=2) as gpool,   # gather buffers
            tc.tile_pool(name="ps", bufs=2, space="PSUM") as ps,
            tc.tile_pool(name="ps3", bufs=2, space="PSUM") as ps3,
            tc.tile_pool(name="psacc", bufs=1, space="PSUM") as psacc,
        ):
            # ---------------- resident loads / constants ----------------
            srcL = res.tile([128, L_low // 16], i16)
            nc.sync.dma_start(srcL[:], P["src_low"][:])
            srcH = res.tile([128, LH_pad // 16], i16)
            nc.sync.dma_start(srcH[:], P["src_high"][:])
            drelL = res.tile([128, L_low // 128], bf16)
            nc.sync.dma_start(drelL[:], P["drel_low"][:])
            drelH = res.tile([128, LH_pad // 128], bf16)
            nc.sync.dma_start(drelH[:], P["drel_high"][:])

            iotaF = res.tile([128, 128], f32)   # value = free index
            nc.gpsimd.iota(iotaF[:], pattern=[[1, 128]], base=0,
                           channel_multiplier=0,
                           allow_small_or_imprecise_dtypes=True)
            iotaC = res.tile([128, 1], f32)    # value = partition index
            nc.gpsimd.iota(iotaC[:], pattern=[[0, 1]], base=0,
                           channel_multiplier=1,
                           allow_small_or_imprecise_dtypes=True)
            ident = res.tile([128, 128], bf16)  # identity (self-loop matmul)
            nc.vector.tensor_scalar(ident[:], iotaF[:], iotaC[:], None,
                                    AL.is_equal)
            onesB = res.tile([128, 1], bf16)
            nc.vector.memset(onesB[:], 1.0)

            # batched-indicator iota, bf16 (values 0..127 exact)
            IB = cfg.IB
            nsubL = L_low // 128
            nsubH = LH_pad // 128
            iotaT = res.tile([128, IB * 128], bf16)
            nc.gpsimd.iota(iotaT[:], pattern=[[0, IB], [1, 128]], base=0,
                           channel_multiplier=0,
                           allow_small_or_imprecise_dtypes=True)

            # weights -> bf16 SBUF
            wg_f = work.tile([128, NL * F], f32, tag="wg_f")
            for l in range(NL):
                nc.sync.dma_start(wg_f[:, l * F:(l + 1) * F], P["Wg"][l])
            wgS = res.tile([128, NL * F], bf16)
            nc.vector.tensor_copy(wgS[:], wg_f[:])

            bg_row = res.tile([1, NL * F], f32)
            nc.sync.dma_start(bg_row[:], P["bg"][:].rearrange("l f -> (l f)"))
            bgB = res.tile([128, NL * F], f32)
            nc.gpsimd.partition_broadcast(bgB[:], bg_row[:])

            w1S = res.tile([128, cfg.H], f32)
            nc.sync.dma_start(w1S[:], P["w1"][:])
            w2S = res.tile([128, 2 * cfg.LD], f32)
            nc.sync.dma_start(w2S[:, 0:cfg.LD], P["w2"][0:128, :])
            nc.sync.dma_start(w2S[:, cfg.LD:2 * cfg.LD], P["w2"][128:256, :])

            b1S = res.tile([128, 2], f32)
            nc.sync.dma_start(b1S[:, 0:1], P["b1"][0:128, :])
            nc.sync.dma_start(b1S[:, 1:2], P["b1"][128:256, :])
            b2_row = res.tile([1, cfg.LD], f32)
            nc.sync.dma_start(b2_row[:], P["b2"][:])
            b2B = res.tile([128, cfg.LD], f32)
            nc.gpsimd.partition_broadcast(b2B[:], b2_row[:])

            xsB = res.tile([128, W * F], bf16)   # local x, node-major
            nc.sync.dma_start(
                xsB[:].rearrange("p (w f) -> p w f", f=F),
                P["xs"][:].rearrange("(w p) f -> p w f", p=128))

            Tbuf = res.tile([128, W * F], bf16)  # dinv*x_l local rows
            SL = res.tile([128, W * F], f32)     # pass-L partial aggT
            degRow = res.tile([1, W * 128], f32)
            dinvS = res.tile([128, W], f32)

            # per-window subchunk schedule, (stream, col) with L first
            schedL, schedH = [], []
            colL = colH = 0
            for w in range(W):
                lst = []
                for _ in range(meta.m_low[w]):
                    lst.append(("L", colL))
                    colL += 1
                schedL.append(lst)
                lst = []
                for _ in range(meta.m_high[w]):
                    lst.append(("H", colH))
                    colH += 1
                schedH.append(lst)

            def make_ind_getter():
                cache = {}

                def get(stream, col):
                    drel, nsub = (drelL, nsubL) if stream == "L" else (drelH, nsubH)
                    s0 = col - col % IB
                    key = (stream, s0)
                    if key not in cache:
                        nb = min(IB, nsub - s0)
                        it = indp.tile([128, IB * 128], bf16, tag="ind")
                        dsl = drel[:, s0:s0 + nb]
                        din = bass.AP(dsl.tensor, dsl.offset,
                                      [list(d) for d in dsl.ap] + [[0, 128]])
                        nc.vector.tensor_tensor(
                            it[:].rearrange("p (c e) -> p c e", e=128)[:, 0:nb, :],
                            din,
                            iotaT[:].rearrange("p (c e) -> p c e", e=128)[:, 0:nb, :],
                            AL.is_equal)
                        cache[key] = it
                        for k in [k for k in cache
                                  if k[0] == stream and k[1] < s0 - IB]:
                            del cache[k]
                    return (cache[key][:]
                            .rearrange("p (c e) -> p c e", e=128)[:, col - s0, :])
                return get

            # gather-call layout per stream
            def gather_calls(L_tot):
                calls = []
                s = 0
                while s < L_tot:
                    n = min(cfg.GCHUNK, L_tot - s)
                    calls.append((s, n))
                    s += n
                return calls
            callsL = gather_calls(L_low)
            callsH = gather_calls(L_high)

            qrot = [0]

            def emit_calls(l, gtiles, stream):
                """Emit all of one stream's gather calls for layer l."""
                elem = 256 if l == 0 else F
                tbl = P["x512"] if l == 0 else tableDs[l % 2]
                srcT = srcL if stream == "L" else srcH
                r0, r1 = ((0, cfg.SPLIT) if stream == "L"
                          else (cfg.SPLIT, TROWS))
                for (s0, n) in (callsL if stream == "L" else callsH):
                    q = qrot[0] % 4
                    qrot[0] += 1
                    gt = gpool.tile([128, (n // 128) * elem], bf16,
                                    tag=f"g{q}")
                    nc.gpsimd.dma_gather(
                        gt[:].rearrange("p (c e) -> p c e", e=elem),
                        tbl[r0:r1, :],
                        srcT[:, s0 // 16:(s0 + n) // 16],
                        n, n, elem, queue_num=q)
                    gtiles[(stream, s0)] = gt

            def gslice(l, gtiles, stream, col):
                calls = callsL if stream == "L" else callsH
                elem = 256 if l == 0 else F
                for (s0, n) in calls:
                    if s0 <= col * 128 < s0 + n:
                        gt = gtiles[(stream, s0)]
                        j = col - s0 // 128
                        return gt[:].rearrange("p (c e) -> p c e", e=elem)[:, j, :]
                raise AssertionError

            # ---------------- degree pass (PE/DVE only) -------------------
            get_ind = make_ind_getter()
            degCol = res.tile([128, W], f32)
            for w in range(W):
                degP = psacc.tile([1, 128], f32, tag="acc1")
                sched_w = schedL[w] + schedH[w]
                n = len(sched_w)
                for i, (stream, col) in enumerate(sched_w):
                    nc.tensor.matmul(degP[:], onesB[:], get_ind(stream, col),
                                     start=(i == 0), stop=(i == n - 1))
                nc.vector.tensor_copy(degRow[:, w * 128:(w + 1) * 128], degP[:])
            nc.sync.dma_start(
                degD[:].rearrange("(a b) -> a b", a=1), degRow[:])
            nc.sync.dma_start(
                degCol[:],
                degD[:].rearrange("(w p) -> p w", p=128))
            sq = work.tile([128, W], f32, tag="sq")
            nc.scalar.activation(sq[:], degCol[:], ACT.Sqrt, bias=1.0)
            nc.vector.reciprocal(dinvS[:], sq[:])

            # degree AllGather (trow order), then dinv patch into x512
            nc.gpsimd.collective_compute(
                "AllGather", mybir.AluOpType.bypass, replica_groups=rg,
                ins=[degD[0:HA]], outs=[degAllD[0:cfg.C * HA]])
            nc.gpsimd.collective_compute(
                "AllGather", mybir.AluOpType.bypass, replica_groups=rg,
                ins=[degD[HA:NPAD]], outs=[degAllD[cfg.C * HA:TROWS]])
            degAllS = work.tile([128, WT], f32, tag="degAllS")
            nc.sync.dma_start(
                degAllS[:], degAllD[:].rearrange("(w p) -> p w", p=128))
            sqA = work.tile([128, WT], f32, tag="sqA")
            nc.scalar.activation(sqA[:], degAllS[:], ACT.Sqrt, bias=1.0)
            dinvA = work.tile([128, WT], f32, tag="dinvA")
            nc.vector.reciprocal(dinvA[:], sqA[:])
            dinvAB = work.tile([128, WT], bf16, tag="dinvAB")
            nc.vector.tensor_copy(dinvAB[:], dinvA[:])
            x512r = P["x512"][:].rearrange("(w p) e -> p w e", p=128)
            for j0 in range(0, WT, 98):
                j1 = min(j0 + 98, WT)
                nc.sync.dma_start(
                    x512r[:, j0:j1, 128:129],
                    dinvAB[:, j0:j1].rearrange("p (w o) -> p w o", o=1))

            # ---------------- layers -------------------------------------
            poolP = None
            cntP = None
            for l in range(NL):
                gtiles = {}
                get_ind = make_ind_getter()
                last = l == NL - 1

                emit_calls(l, gtiles, "L")
                if l == 0:
                    emit_calls(l, gtiles, "H")

                def lhs_of(stream, col):
                    g = gslice(l, gtiles, stream, col)
                    if l == 0:
                        gs = work.tile([128, F], bf16, tag="gs")
                        xpart = bass.AP(g.tensor, g.offset,
                                        [list(g.ap[0]), [1, F]])
                        dcol = bass.AP(g.tensor, g.offset + F,
                                       [list(g.ap[0]), [0, F]])
                        nc.vector.tensor_tensor(gs[:], xpart, dcol, AL.mult)
                        return gs[:]
                    return g

                # ---- pass L: low-stream partial aggregates -> SL ----
                for w in range(W):
                    aggLP = ps3.tile([128, F], f32, tag="aggL")
                    n = len(schedL[w])
                    for i, (stream, col) in enumerate(schedL[w]):
                        nc.tensor.matmul(aggLP[:], lhs_of(stream, col),
                                         get_ind(stream, col),
                                         start=(i == 0), stop=(i == n - 1))
                    nc.vector.tensor_copy(SL[:, w * F:(w + 1) * F], aggLP[:])

                if l > 0:
                    # half-B table AG for this layer was deferred until after
                    # our L-stream gather calls; emit H calls now
                    emit_calls(l, gtiles, "H")

                if last:
                    poolP = psacc.tile([128, GR], f32, tag="poolP")
                    cntP = psacc.tile([1, GR], f32, tag="acc1")

                # ---- pass H: high stream + self loop + epilogue ----
                for w in range(W):
                    if l == 0:
                        # local table rows dinv*x for the self-loop matmul
                        nc.vector.tensor_scalar(
                            Tbuf[:, w * F:(w + 1) * F],
                            xsB[:, w * F:(w + 1) * F],
                            dinvS[:, w:w + 1], None, AL.mult)
                    aggP = ps3.tile([128, F], f32, tag="aggH")
                    n = len(schedH[w]) + 1
                    for i, (stream, col) in enumerate(schedH[w]):
                        nc.tensor.matmul(aggP[:], lhs_of(stream, col),
                                         get_ind(stream, col),
                                         start=(i == 0), stop=False)
                    nc.tensor.matmul(aggP[:], Tbuf[:, w * F:(w + 1) * F],
                                     ident[:],
                                     start=(len(schedH[w]) == 0), stop=True)
                    aT = work.tile([128, F], bf16, tag="aT")
                    nc.vector.tensor_tensor(aT[:], aggP[:],
                                            SL[:, w * F:(w + 1) * F], AL.add)
                    xP = ps.tile([128, F], f32, tag="xP")
                    nc.tensor.matmul(xP[:], aT[:], wgS[:, l * F:(l + 1) * F],
                                     start=True, stop=True)
                    v = work.tile([128, F], f32, tag="v")
                    nc.vector.scalar_tensor_tensor(
                        v[:], xP[:], dinvS[:, w:w + 1],
                        bgB[:, l * F:(l + 1) * F], AL.mult, AL.add)
                    xn = work.tile([128, F], bf16, tag="xn")
                    nc.scalar.activation(xn[:], v[:], ACT.Relu)

                    if not last:
                        # next-layer local table rows (also the AG shard)
                        nc.vector.tensor_scalar(
                            Tbuf[:, w * F:(w + 1) * F], xn[:],
                            dinvS[:, w:w + 1], None, AL.mult)
                        if w == WA - 1:
                            shardD, tableD = shardDs[(l + 1) % 2], tableDs[(l + 1) % 2]
                            nc.sync.dma_start(
                                shardD[0:HA].rearrange("(w p) f -> p w f", p=128),
                                Tbuf[:, 0:HA * F // 128]
                                .rearrange("p (w f) -> p w f", f=F))
                            nc.gpsimd.collective_compute(
                                "AllGather", mybir.AluOpType.bypass,
                                replica_groups=rg,
                                ins=[shardD[0:HA]],
                                outs=[tableD[0:cfg.C * HA]])
                    else:
                        pw = work.tile([128, GR], bf16, tag="pw")
                        nc.sync.dma_start(
                            pw[:],
                            P["pind"][w * 128:(w + 1) * 128, :])
                        nc.tensor.matmul(
                            poolP[:], xn[:], pw[:],
                            start=(w == 0), stop=(w == W - 1),
                            skip_group_check=True)
                        nc.tensor.matmul(
                            cntP[:], onesB[:], pw[:],
                            start=(w == 0), stop=(w == W - 1),
                            skip_group_check=True)

                if not last:
                    shardD, tableD = shardDs[(l + 1) % 2], tableDs[(l + 1) % 2]
                    nc.sync.dma_start(
                        shardD[HA:NPAD].rearrange("(w p) f -> p w f", p=128),
                        Tbuf[:, HA * F // 128:NPAD * F // 128]
                        .rearrange("p (w f) -> p w f", f=F))
                    nc.gpsimd.collective_compute(
                        "AllGather", mybir.AluOpType.bypass, replica_groups=rg,
                        ins=[shardD[HA:NPAD]],
                        outs=[tableD[cfg.C * HA:TROWS]])

            # ---------------- pooling allreduce + head ----------------
            sumsS = work.tile([128, GR], f32, tag="sumsS")
            nc.vector.tensor_copy(sumsS[:], poolP[:])
            cntS = work.tile([1, GR], f32, tag="cntS")
            nc.vector.tensor_copy(cntS[:], cntP[:])
            nc.sync.dma_start(arInD[0:128, :], sumsS[:])
            nc.sync.dma_start(arInD[128:129, :], cntS[:])
            nc.gpsimd.collective_compute(
                "AllReduce", mybir.AluOpType.add, replica_groups=rg,
                ins=[arInD[:]], outs=[arOutD[:]])
            sumsA = work.tile([128, GR], f32, tag="sumsA")
            nc.sync.dma_start(sumsA[:], arOutD[0:128, :])
            cntA = work.tile([1, GR], f32, tag="cntA")
            nc.sync.dma_start(cntA[:], arOutD[128:129, :])
            cntM = work.tile([1, GR], f32, tag="cntM")
            nc.vector.tensor_scalar(cntM[:], cntA[:], 1.0, None, AL.max)
            rec = work.tile([1, GR], f32, tag="rec")
            nc.vector.reciprocal(rec[:], cntM[:])
            recB = work.tile([128, GR], f32, tag="recB")
            nc.gpsimd.partition_broadcast(recB[:], rec[:])
            pooledT = work.tile([128, GR], f32, tag="pooledT")
            nc.vector.tensor_tensor(pooledT[:], sumsA[:], recB[:], AL.mult)

            h1 = []
            for h in range(2):
                h1P = ps3.tile([128, GR], f32, tag="aggL")
                nc.tensor.matmul(h1P[:], w1S[:, h * 128:(h + 1) * 128],
                                 pooledT[:], start=True, stop=True)
                h1S = work.tile([128, GR], f32, tag=f"h1S{h}")
                nc.scalar.activation(h1S[:], h1P[:], ACT.Relu,
                                     bias=b1S[:, h:h + 1])
                h1.append(h1S)
            for g in range(GR // 128):
                oP = ps3.tile([128, cfg.LD], f32, tag="aggH")
                nc.tensor.matmul(oP[:], h1[0][:, g * 128:(g + 1) * 128],
                                 w2S[:, 0:cfg.LD], start=True, stop=False)
                nc.tensor.matmul(oP[:], h1[1][:, g * 128:(g + 1) * 128],
                                 w2S[:, cfg.LD:2 * cfg.LD], start=False, stop=True)
                oS = work.tile([128, cfg.LD], f32, tag="oS")
                nc.vector.tensor_tensor(oS[:], oP[:], b2B[:], AL.add)
                nc.sync.dma_start(out_ext[g * 128:(g + 1) * 128, :], oS[:])

    nc.compile()
    return nc


# ---------------------------------------------------------------------------
# Entry point
# ---------------------------------------------------------------------------
_CACHE = {}


def _build(cfg, meta):
    key = (tuple(meta.m_low), tuple(meta.m_high))
    if key not in _CACHE:
        _CACHE[key] = build_graph(cfg, meta)
    return _CACHE[key]


def kernel(**inputs) -> np.ndarray:
    from concourse.bass_utils import run_bass_kernel_spmd
    cfg = Cfg()
    meta = host_prep(cfg, **inputs)
    nc = _build(cfg, meta)
    res = run_bass_kernel_spmd(nc, meta.in_maps, list(range(cfg.C)))
    return np.asarray(res.results[0]["out"], dtype=np.float32)


# revision 17
# speedup vs baseline: 1.0416x; 1.0416x over previous
"""GCN (3-layer message passing + mean-pool + MLP head) on 8 Trainium2 NeuronCores.

v2.1: aggregate-then-matmul formulation.  Per layer the table holds
dinv-scaled features; each core dma_gathers its edges' src rows with 4-way
SWDGE queue rotation (descriptor generation parallelized over all four Q7
core pairs) and segment-sums them feature-major via indicator matmuls
(aggT = sum_k G_k-lhsT x Ind_k); the self-loop is one identity-matmul per
window against SBUF-resident local table rows, and the per-window W-matmul
runs post-aggregation (no transposes anywhere).  The GCN bias enters as a
rank-1 matmul into the same PSUM tile, so the whole epilogue runs on the
otherwise-idle Scalar engine.  Indicator one-hots are built once on DVE,
round-tripped through DRAM, and streamed back each layer.  Layer 1's table
is built locally (tiny degree AllGather + scale of the replicated input),
so only two 12.8MB feature AllGathers remain, each half-split and
overlapped with the opposite half's compute via an L/H two-pass window
loop.  Pooling is an indicator matmul + [129,256] AllReduce; the MLP head
is computed redundantly."""

import numpy as np
from dataclasses import dataclass, field


# ---------------------------------------------------------------------------
# Config
# ---------------------------------------------------------------------------
@dataclass
class Cfg:
    N: int = 50000          # nodes
    E: int = 600000         # edges
    F: int = 128            # feature dim
    NL: int = 3             # gcn layers
    G: int = 256            # graphs
    H: int = 256            # hidden dim of head
    LD: int = 2             # label dim
    C: int = 8              # cores
    GCHUNK: int = 1024      # edges per dma_gather call
    IB: int = 8             # indicator subchunks built per DVE op

    @property
    def NPC(self):          # nodes per core
        return self.N // self.C

    @property
    def W(self):            # 128-node windows per core
        return (self.NPC + 127) // 128

    @property
    def NPAD(self):         # padded nodes per core
        return self.W * 128

    @property
    def TROWS(self):        # gather-table rows
        return self.C * self.NPAD

    @property
    def HA(self):           # local rows in table half A (window-aligned)
        return 128 * ((self.W + 1) // 2)

    @property
    def HB(self):           # local rows in table half B
        return self.NPAD - self.HA

    @property
    def SPLIT(self):        # low/high gather-stream boundary = half-A rows
        return self.C * self.HA


@dataclass
class Meta:
    """Uniform (core-independent) graph structure + per-core data arrays."""
    m_low: list = field(default_factory=list)    # per-window low subchunk count
    m_high: list = field(default_factory=list)   # per-window high subchunk count
    L_low: int = 0
    L_high: int = 0
    in_maps: list = field(default_factory=list)  # per-core tensor dicts


# ---------------------------------------------------------------------------
# Host-side sharding / layout prep (pure numpy, no model math)
# ---------------------------------------------------------------------------
def _wrap16(arr_i16):
    # slot i -> [i % 16, i // 16]; 16-row wrap replicated to 128 partitions
    # (one copy per GPSIMD Q7 core).
    return np.ascontiguousarray(np.tile(arr_i16.reshape(-1, 16).T, (8, 1)))


def _wrap128(arr):
    # slot i -> [i % 128, i // 128]
    return np.ascontiguousarray(arr.reshape(-1, 128).T)


def host_prep(cfg: Cfg, x, Wg, bg, w1, b1, w2, b2, edge_index, batch) -> Meta:
    import ml_dtypes
    C, NPC, W, NPAD = cfg.C, cfg.NPC, cfg.W, cfg.NPAD
    src = np.asarray(edge_index[0], dtype=np.int64)
    dst = np.asarray(edge_index[1], dtype=np.int64)
    batch = np.asarray(batch, dtype=np.int64)
    x = np.asarray(x, dtype=np.float32)

    # table row of a global node id: rows [0, C*HA) hold every core's first
    # HA local rows, rows [C*HA, TROWS) the remaining HB.
    HA, HB = cfg.HA, cfg.HB
    nid = np.arange(cfg.N, dtype=np.int64)
    nc_, nl = nid // NPC, nid % NPC
    trow_of = np.where(nl < HA, nc_ * HA + nl, C * HA + nc_ * HB + (nl - HA))
    trow = trow_of[src]

    # replicated node features in table-row order (layer-1 table source)
    xfull = np.zeros((cfg.TROWS, cfg.F), dtype=ml_dtypes.bfloat16)
    xfull[trow_of] = x.astype(ml_dtypes.bfloat16)

    # per (core, window, half) edge lists
    per_core = []
    for c in range(C):
        m = (dst // NPC) == c
        s_c, d_c, t_c = src[m], dst[m], trow[m]
        dloc = d_c - c * NPC
        order = np.argsort(dloc, kind="stable")
        s_c, dloc, t_c = s_c[order], dloc[order], t_c[order]
        win = dloc // 128
        drel = dloc - win * 128
        lowm = t_c < cfg.SPLIT
        lists = []
        for w in range(W):
            wm = win == w
            lists.append((
                (t_c[wm & lowm], drel[wm & lowm]),
                (t_c[wm & ~lowm] - cfg.SPLIT, drel[wm & ~lowm]),
            ))
        per_core.append(lists)

    # uniform subchunk counts (max over cores), >=1 low subchunk per window
    m_low = [max(1, max(-(-len(per_core[c][w][0][0]) // 128) for c in range(C)))
             for w in range(W)]
    m_high = [max(-(-len(per_core[c][w][1][0]) // 128) for c in range(C))
              for w in range(W)]
    L_low = 128 * sum(m_low)
    L_high = 128 * sum(m_high)

    meta = Meta(m_low=m_low, m_high=m_high, L_low=L_low, L_high=L_high)

    for c in range(C):
        idx_low = np.zeros(L_low, np.int16)
        drel_low = np.full(L_low, -1.0, np.float32)
        idx_high = np.zeros(max(L_high, 128), np.int16)
        drel_high = np.full(max(L_high, 128), -1.0, np.float32)
        ol = oh = 0
        for w in range(W):
            (tl, dl), (th, dh) = per_core[c][w]
            idx_low[ol:ol + len(tl)] = tl.astype(np.int16)
            drel_low[ol:ol + len(dl)] = dl.astype(np.float32)
            ol += 128 * m_low[w]
            idx_high[oh:oh + len(th)] = th.astype(np.int16)
            drel_high[oh:oh + len(dh)] = dh.astype(np.float32)
            oh += 128 * m_high[w]
        assert ol == L_low and oh == L_high

        # local x shard, node-major bf16 (layer-1 self-loop table rows)
        xs = np.zeros((NPAD, cfg.F), ml_dtypes.bfloat16)
        xs[:NPC] = x[c * NPC:(c + 1) * NPC].astype(ml_dtypes.bfloat16)

        # pooling one-hot indicators [node-slot x G], bf16
        pind = np.zeros((NPAD, cfg.G), ml_dtypes.bfloat16)
        pind[np.arange(NPC), batch[c * NPC:(c + 1) * NPC]] = 1.0

        meta.in_maps.append(dict(
            xs=np.ascontiguousarray(xs),
            xfull=xfull,
            pind=np.ascontiguousarray(pind),
            src_low=_wrap16(idx_low),
            src_high=_wrap16(idx_high),
            drel_low=_wrap128(drel_low),
            drel_high=_wrap128(drel_high),
            Wg=np.asarray(Wg, np.float32),
            bg=np.asarray(bg, np.float32),
            w1=np.asarray(w1, np.float32),
            b1=np.asarray(b1, np.float32).reshape(cfg.H, 1),
            w2=np.asarray(w2, np.float32),
            b2=np.asarray(b2, np.float32).reshape(1, cfg.LD),
        ))
    return meta


# ---------------------------------------------------------------------------
# Device graph
# ---------------------------------------------------------------------------
def build_graph(cfg: Cfg, meta: Meta):
    import concourse.bass as bass
    import concourse.bacc as bacc
    import concourse.mybir as mybir
    import concourse.tile as tile

    F, W, NL, NPAD = cfg.F, cfg.W, cfg.NL, cfg.NPAD
    GR = cfg.G
    f32, bf16, i16 = mybir.dt.float32, mybir.dt.bfloat16, mybir.dt.int16
    AL = mybir.AluOpType
    ACT = mybir.ActivationFunctionType
    L_low, L_high = meta.L_low, meta.L_high
    LH_pad = max(L_high, 128)
    HA, WA = cfg.HA, cfg.HA // 128
    TROWS = cfg.TROWS
    WT = TROWS // 128
    WTA = cfg.C * HA // 128          # table windows in half A
    nsubL = L_low // 128
    nsubH = LH_pad // 128

    nc = bacc.Bacc("TRN2", target_bir_lowering=False, debug=False,
                   num_devices=cfg.C, num_swdge_queues=4)

    # --- external IO ------------------------------------------------------
    P = {}
    P["xs"] = nc.declare_dram_parameter("xs", [NPAD, F], bf16, isOutput=False)
    P["xfull"] = nc.declare_dram_parameter("xfull", [TROWS, F], bf16, isOutput=False)
    P["pind"] = nc.declare_dram_parameter("pind", [NPAD, GR], bf16, isOutput=False)
    P["src_low"] = nc.declare_dram_parameter("src_low", [128, L_low // 16], i16, isOutput=False)
    P["src_high"] = nc.declare_dram_parameter("src_high", [128, LH_pad // 16], i16, isOutput=False)
    P["drel_low"] = nc.declare_dram_parameter("drel_low", [128, nsubL], f32, isOutput=False)
    P["drel_high"] = nc.declare_dram_parameter("drel_high", [128, nsubH], f32, isOutput=False)
    P["Wg"] = nc.declare_dram_parameter("Wg", [NL, F, F], f32, isOutput=False)
    P["bg"] = nc.declare_dram_parameter("bg", [NL, F], f32, isOutput=False)
    P["w1"] = nc.declare_dram_parameter("w1", [F, cfg.H], f32, isOutput=False)
    P["b1"] = nc.declare_dram_parameter("b1", [cfg.H, 1], f32, isOutput=False)
    P["w2"] = nc.declare_dram_parameter("w2", [cfg.H, cfg.LD], f32, isOutput=False)
    P["b2"] = nc.declare_dram_parameter("b2", [1, cfg.LD], f32, isOutput=False)
    out_ext = nc.declare_dram_parameter("out", [GR, cfg.LD], f32, isOutput=True)

    # --- internal DRAM ----------------------------------------------------
    tableX = nc.dram_tensor("tableX", [TROWS, F], bf16)          # layer-1
    tableDs = [nc.dram_tensor(f"tableD{i}", [TROWS, F], bf16,
                              addr_space="Shared") for i in range(2)]
    shardDs = [nc.dram_tensor(f"shardD{i}", [NPAD, F], bf16) for i in range(2)]
    NSUB = nsubL + nsubH
    indD = nc.dram_tensor("indD", [128, NSUB * 128], bf16)       # one-hots
    degD = nc.dram_tensor("degD", [NPAD], f32)
    degAllD = nc.dram_tensor("degAllD", [TROWS], f32, addr_space="Shared")
    arInD = nc.dram_tensor("arInD", [129, GR], f32)
    arOutD = nc.dram_tensor("arOutD", [129, GR], f32, addr_space="Shared")

    rg = [list(range(cfg.C))]

    with tile.TileContext(nc) as tc:
        with (
            tc.tile_pool(name="res", bufs=1) as res,      # resident tensors
            tc.tile_pool(name="work", bufs=3) as work,    # per-window temps
            tc.tile_pool(name="indp", bufs=6) as indp,    # indicator batches
            tc.tile_pool(name="gbuf", bufs=2) as gpool,   # gather buffers
            tc.tile_pool(name="ps", bufs=2, space="PSUM") as ps,
            tc.tile_pool(name="ps3", bufs=2, space="PSUM") as ps3,
            tc.tile_pool(name="psacc", bufs=1, space="PSUM") as psacc,
        ):
            # ---------------- resident loads / constants ----------------
            srcL = res.tile([128, L_low // 16], i16)
            nc.sync.dma_start(srcL[:], P["src_low"][:])
            srcH = res.tile([128, LH_pad // 16], i16)
            nc.sync.dma_start(srcH[:], P["src_high"][:])
            drelL = res.tile([128, nsubL], f32)
            nc.sync.dma_start(drelL[:], P["drel_low"][:])
            drelH = res.tile([128, nsubH], f32)
            nc.sync.dma_start(drelH[:], P["drel_high"][:])

            iotaF = res.tile([128, 128], f32)   # value = free index
            nc.gpsimd.iota(iotaF[:], pattern=[[1, 128]], base=0,
                           channel_multiplier=0,
                           allow_small_or_imprecise_dtypes=True)
            iotaC = res.tile([128, 1], f32)    # value = partition index
            nc.gpsimd.iota(iotaC[:], pattern=[[0, 1]], base=0,
                           channel_multiplier=1,
                           allow_small_or_imprecise_dtypes=True)
            ident = res.tile([128, 128], bf16)  # identity (self-loop matmul)
            nc.vector.tensor_scalar(ident[:], iotaF[:], iotaC[:], None,
                                    AL.is_equal)
            onesB = res.tile([128, 1], bf16)
            nc.vector.memset(onesB[:], 1.0)
            onesRow = res.tile([1, 128], bf16)
            nc.vector.memset(onesRow[:], 1.0)

            IB = cfg.IB
            iotaT = res.tile([128, IB * 128], f32)
            nc.gpsimd.iota(iotaT[:], pattern=[[0, IB], [1, 128]], base=0,
                           channel_multiplier=0,
                           allow_small_or_imprecise_dtypes=True)

            # weights -> bf16 SBUF
            wg_f = work.tile([128, NL * F], f32, tag="wg_f")
            for l in range(NL):
                nc.sync.dma_start(wg_f[:, l * F:(l + 1) * F], P["Wg"][l])
            wgS = res.tile([128, NL * F], bf16)
            nc.vector.tensor_copy(wgS[:], wg_f[:])

            bg_row = res.tile([1, NL * F], f32)
            nc.sync.dma_start(bg_row[:], P["bg"][:].rearrange("l f -> (l f)"))
            bgRowB = res.tile([1, NL * F], bf16)   # rank-1 bias matmul rhs
            nc.vector.tensor_copy(bgRowB[:], bg_row[:])

            w1S = res.tile([128, cfg.H], f32)
            nc.sync.dma_start(w1S[:], P["w1"][:])
            w2S = res.tile([128, 2 * cfg.LD], f32)
            nc.sync.dma_start(w2S[:, 0:cfg.LD], P["w2"][0:128, :])
            nc.sync.dma_start(w2S[:, cfg.LD:2 * cfg.LD], P["w2"][128:256, :])

            b1S = res.tile([128, 2], f32)
            nc.sync.dma_start(b1S[:, 0:1], P["b1"][0:128, :])
            nc.sync.dma_start(b1S[:, 1:2], P["b1"][128:256, :])
            b2_row = res.tile([1, cfg.LD], f32)
            nc.sync.dma_start(b2_row[:], P["b2"][:])
            b2B = res.tile([128, cfg.LD], f32)
            nc.gpsimd.partition_broadcast(b2B[:], b2_row[:])

            xsB = res.tile([128, W * F], bf16)   # local x, node-major
            nc.sync.dma_start(
                xsB[:].rearrange("p (w f) -> p w f", f=F),
                P["xs"][:].rearrange("(w p) f -> p w f", p=128))

            Tbuf = res.tile([128, W * F], bf16)  # dinv*x_l local rows
            SL = res.tile([128, W * F], f32)     # pass-L partial aggT
            degRow = res.tile([1, W * 128], f32)
            dinvS = res.tile([128, W], f32)
            dinvA = res.tile([128, WT], f32)     # table-row dinv (all cores)

            # per-window subchunk schedule, split by stream
            schedL, schedH = [], []
            colL = colH = 0
            for w in range(W):
                lst = []
                for _ in range(meta.m_low[w]):
                    lst.append(("L", colL))
                    colL += 1
                schedL.append(lst)
                lst = []
                for _ in range(meta.m_high[w]):
                    lst.append(("H", colH))
                    colH += 1
                schedH.append(lst)

            def flat_col(stream, col):
                return col if stream == "L" else nsubL + col

            def make_ind_builder():
                """JIT builder (deg pass): builds batches on DVE and writes
                each batch to indD for later streaming."""
                cache = {}

                def get(stream, col):
                    drel, nsub = (drelL, nsubL) if stream == "L" else (drelH, nsubH)
                    s0 = col - col % IB
                    key = (stream, s0)
                    if key not in cache:
                        nb = min(IB, nsub - s0)
                        it = indp.tile([128, IB * 128], bf16, tag="ind")
                        dsl = drel[:, s0:s0 + nb]
                        din = bass.AP(dsl.tensor, dsl.offset,
                                      [list(d) for d in dsl.ap] + [[0, 128]])
                        nc.vector.tensor_tensor(
                            it[:].rearrange("p (c e) -> p c e", e=128)[:, 0:nb, :],
                            din,
                            iotaT[:].rearrange("p (c e) -> p c e", e=128)[:, 0:nb, :],
                            AL.is_equal)
                        fc = flat_col(stream, s0)
                        nc.sync.dma_start(
                            indD[:, fc * 128:(fc + nb) * 128],
                            it[:, 0:nb * 128])
                        cache[key] = it
                        for k in [k for k in cache
                                  if k[0] == stream and k[1] < s0 - IB]:
                            del cache[k]
                    return (cache[key][:]
                            .rearrange("p (c e) -> p c e", e=128)[:, col - s0, :])
                return get

            def make_ind_streamer():
                """Per-layer getter: streams prebuilt one-hots from indD."""
                cache = {}

                def get(stream, col):
                    nsub = nsubL if stream == "L" else nsubH
                    s0 = col - col % IB
                    key = (stream, s0)
                    if key not in cache:
                        nb = min(IB, nsub - s0)
                        it = indp.tile([128, IB * 128], bf16, tag="ind")
                        fc = flat_col(stream, s0)
                        nc.sync.dma_start(
                            it[:, 0:nb * 128],
                            indD[:, fc * 128:(fc + nb) * 128])
                        cache[key] = it
                        for k in [k for k in cache
                                  if k[0] == stream and k[1] < s0 - IB]:
                            del cache[k]
                    return (cache[key][:]
                            .rearrange("p (c e) -> p c e", e=128)[:, col - s0, :])
                return get

            # gather-call layout per stream
            def gather_calls(L_tot):
                calls = []
                s = 0
                while s < L_tot:
                    n = min(cfg.GCHUNK, L_tot - s)
                    calls.append((s, n))
                    s += n
                return calls
            callsL = gather_calls(L_low)
            callsH = gather_calls(L_high)

            qrot = [0]

            def emit_calls(l, gtiles, stream):
                tbl = tableX if l == 0 else tableDs[l % 2]
                srcT = srcL if stream == "L" else srcH
                r0, r1 = ((0, cfg.SPLIT) if stream == "L"
                          else (cfg.SPLIT, TROWS))
                for (s0, n) in (callsL if stream == "L" else callsH):
                    q = qrot[0] % 4
                    qrot[0] += 1
                    gt = gpool.tile([128, (n // 128) * F], bf16, tag=f"g{q}")
                    nc.gpsimd.dma_gather(
                        gt[:].rearrange("p (c e) -> p c e", e=F),
                        tbl[r0:r1, :],
                        srcT[:, s0 // 16:(s0 + n) // 16],
                        n, n, F, queue_num=q)
                    gtiles[(stream, s0)] = gt

            def gslice(gtiles, stream, col):
                calls = callsL if stream == "L" else callsH
                for (s0, n) in calls:
                    if s0 <= col * 128 < s0 + n:
                        gt = gtiles[(stream, s0)]
                        j = col - s0 // 128
                        return gt[:].rearrange("p (c e) -> p c e", e=F)[:, j, :]
                raise AssertionError

            # ---------------- degree pass + layer-1 table ----------------
            # per half: deg windows -> deg AG -> dinv -> scaled table build
            get_ind = make_ind_builder()
            for w in range(W):
                degP = ps3.tile([1, 128], f32, tag="aggL")
                sched_w = schedL[w] + schedH[w]
                n = len(sched_w)
                for i, (stream, col) in enumerate(sched_w):
                    nc.tensor.matmul(degP[:], onesB[:], get_ind(stream, col),
                                     start=(i == 0), stop=(i == n - 1))
                nc.scalar.activation(degRow[:, w * 128:(w + 1) * 128],
                                     degP[:], ACT.Copy)
                if w == WA - 1:
                    nc.sync.dma_start(
                        degD[0:HA].rearrange("(a b) -> a b", a=1),
                        degRow[:, 0:HA])
                    nc.gpsimd.collective_compute(
                        "AllGather", mybir.AluOpType.bypass, replica_groups=rg,
                        ins=[degD[0:HA]], outs=[degAllD[0:cfg.C * HA]])
            nc.sync.dma_start(
                degD[HA:NPAD].rearrange("(a b) -> a b", a=1),
                degRow[:, HA:NPAD])
            nc.gpsimd.collective_compute(
                "AllGather", mybir.AluOpType.bypass, replica_groups=rg,
                ins=[degD[HA:NPAD]], outs=[degAllD[cfg.C * HA:TROWS]])

            # local dinv (epilogue scaling)
            degCol = res.tile([128, W], f32)
            nc.sync.dma_start(
                degCol[:], degD[:].rearrange("(w p) -> p w", p=128))
            sq = work.tile([128, W], f32, tag="sq")
            nc.scalar.activation(sq[:], degCol[:], ACT.Sqrt, bias=1.0)
            nc.vector.reciprocal(dinvS[:], sq[:])

            # per-half: table-row dinv + scaled layer-1 table build
            TB = 8   # table windows per build chunk
            for (j0, j1) in ((0, WTA), (WTA, WT)):
                degAllS = work.tile([128, WT], f32, tag="degAllS")
                nc.sync.dma_start(
                    degAllS[:, j0:j1],
                    degAllD[j0 * 128:j1 * 128].rearrange("(w p) -> p w", p=128))
                sqA = work.tile([128, WT], f32, tag="sqA")
                nc.scalar.activation(sqA[:, j0:j1], degAllS[:, j0:j1],
                                     ACT.Sqrt, bias=1.0)
                nc.vector.reciprocal(dinvA[:, j0:j1], sqA[:, j0:j1])
                for t0 in range(j0, j1, TB):
                    t1 = min(t0 + TB, j1)
                    xbt = work.tile([128, TB * F], bf16, tag="xbt")
                    nc.sync.dma_start(
                        xbt[:, 0:(t1 - t0) * F].rearrange("p (w f) -> p w f", f=F),
                        P["xfull"][t0 * 128:t1 * 128, :]
                        .rearrange("(w p) f -> p w f", p=128))
                    tbt = work.tile([128, TB * F], bf16, tag="tbt")
                    for t in range(t0, t1):
                        nc.scalar.activation(
                            tbt[:, (t - t0) * F:(t - t0 + 1) * F],
                            xbt[:, (t - t0) * F:(t - t0 + 1) * F],
                            ACT.Copy, scale=dinvA[:, t:t + 1])
                    nc.sync.dma_start(
                        tableX[t0 * 128:t1 * 128, :]
                        .rearrange("(w p) f -> p w f", p=128),
                        tbt[:, 0:(t1 - t0) * F].rearrange("p (w f) -> p w f", f=F))

            # ---------------- layers -------------------------------------
            poolP = None
            cntP = None
            for l in range(NL):
                gtiles = {}
                get_ind = make_ind_streamer()
                last = l == NL - 1

                emit_calls(l, gtiles, "L")
                if l == 0:
                    emit_calls(l, gtiles, "H")

                # ---- pass L: low-stream partial aggregates -> SL ----
                for w in range(W):
                    aggLP = ps3.tile([128, F], f32, tag="aggL")
                    n = len(schedL[w])
                    for i, (stream, col) in enumerate(schedL[w]):
                        nc.tensor.matmul(aggLP[:], gslice(gtiles, stream, col),
                                         get_ind(stream, col),
                                         start=(i == 0), stop=(i == n - 1))
                    nc.scalar.activation(SL[:, w * F:(w + 1) * F], aggLP[:],
                                         ACT.Copy)

                if l > 0:
                    emit_calls(l, gtiles, "H")

                if last:
                    poolP = psacc.tile([128, GR], f32, tag="poolP")
                    cntP = psacc.tile([1, GR], f32, tag="acc1")

                # ---- pass H: high stream + self loop + epilogue ----
                for w in range(W):
                    if l == 0:
                        nc.scalar.activation(
                            Tbuf[:, w * F:(w + 1) * F],
                            xsB[:, w * F:(w + 1) * F],
                            ACT.Copy, scale=dinvS[:, w:w + 1])
                    aggP = ps3.tile([128, F], f32, tag="aggH")
                    for i, (stream, col) in enumerate(schedH[w]):
                        nc.tensor.matmul(aggP[:], gslice(gtiles, stream, col),
                                         get_ind(stream, col),
                                         start=(i == 0), stop=False)
                    nc.tensor.matmul(aggP[:], Tbuf[:, w * F:(w + 1) * F],
                                     ident[:],
                                     start=(len(schedH[w]) == 0), stop=True)
                    aT = work.tile([128, F], bf16, tag="aT")
                    nc.vector.tensor_tensor(aT[:], aggP[:],
                                            SL[:, w * F:(w + 1) * F], AL.add)
                    xP = ps.tile([128, F], f32, tag="xP")
                    nc.tensor.matmul(xP[:], aT[:], wgS[:, l * F:(l + 1) * F],
                                     start=True, stop=False)
                    nc.tensor.matmul(xP[:], onesRow[:],
                                     bgRowB[:, l * F:(l + 1) * F],
                                     start=False, stop=True)
                    xn = work.tile([128, F], bf16, tag="xn")
                    nc.scalar.activation(xn[:], xP[:], ACT.Relu,
                                         scale=dinvS[:, w:w + 1])

                    if not last:
                        nc.scalar.activation(
                            Tbuf[:, w * F:(w + 1) * F], xn[:],
                            ACT.Copy, scale=dinvS[:, w:w + 1])
                        if w == WA - 1:
                            shardD = shardDs[(l + 1) % 2]
                            tableD = tableDs[(l + 1) % 2]
                            nc.sync.dma_start(
                                shardD[0:HA].rearrange("(w p) f -> p w f", p=128),
                                Tbuf[:, 0:HA * F // 128]
                                .rearrange("p (w f) -> p w f", f=F))
                            nc.gpsimd.collective_compute(
                                "AllGather", mybir.AluOpType.bypass,
                                replica_groups=rg,
                                ins=[shardD[0:HA]],
                                outs=[tableD[0:cfg.C * HA]])
                    else:
                        pw = work.tile([128, GR], bf16, tag="pw")
                        nc.sync.dma_start(
                            pw[:], P["pind"][w * 128:(w + 1) * 128, :])
                        nc.tensor.matmul(
                            poolP[:], xn[:], pw[:],
                            start=(w == 0), stop=(w == W - 1),
                            skip_group_check=True)
                        nc.tensor.matmul(
                            cntP[:], onesB[:], pw[:],
                            start=(w == 0), stop=(w == W - 1),
                            skip_group_check=True)

                if not last:
                    shardD = shardDs[(l + 1) % 2]
                    tableD = tableDs[(l + 1) % 2]
                    nc.sync.dma_start(
                        shardD[HA:NPAD].rearrange("(w p) f -> p w f", p=128),
                        Tbuf[:, HA * F // 128:NPAD * F // 128]
                        .rearrange("p (w f) -> p w f", f=F))
                    nc.gpsimd.collective_compute(
                        "AllGather", mybir.AluOpType.bypass, replica_groups=rg,
                        ins=[shardD[HA:NPAD]],
                        outs=[tableD[cfg.C * HA:TROWS]])

            # ---------------- pooling allreduce + head ----------------
            sumsS = work.tile([128, GR], f32, tag="sumsS")
            nc.vector.tensor_copy(sumsS[:], poolP[:])
            cntS = work.tile([1, GR], f32, tag="cntS")
            nc.vector.tensor_copy(cntS[:], cntP[:])
            nc.sync.dma_start(arInD[0:128, :], sumsS[:])
            nc.sync.dma_start(arInD[128:129, :], cntS[:])
            nc.gpsimd.collective_compute(
                "AllReduce", mybir.AluOpType.add, replica_groups=rg,
                ins=[arInD[:]], outs=[arOutD[:]])
            sumsA = work.tile([128, GR], f32, tag="sumsA")
            nc.sync.dma_start(sumsA[:], arOutD[0:128, :])
            cntA = work.tile([1, GR], f32, tag="cntA")
            nc.sync.dma_start(cntA[:], arOutD[128:129, :])
            cntM = work.tile([1, GR], f32, tag="cntM")
            nc.vector.tensor_scalar(cntM[:], cntA[:], 1.0, None, AL.max)
            rec = work.tile([1, GR], f32, tag="rec")
            nc.vector.reciprocal(rec[:], cntM[:])
            recB = work.tile([128, GR], f32, tag="recB")
            nc.gpsimd.partition_broadcast(recB[:], rec[:])
            pooledT = work.tile([128, GR], f32, tag="pooledT")
            nc.vector.tensor_tensor(pooledT[:], sumsA[:], recB[:], AL.mult)

            h1 = []
            for h in range(2):
                h1P = ps3.tile([128, GR], f32, tag="aggL")
                nc.tensor.matmul(h1P[:], w1S[:, h * 128:(h + 1) * 128],
                                 pooledT[:], start=True, stop=True)
                h1S = work.tile([128, GR], f32, tag=f"h1S{h}")
                nc.scalar.activation(h1S[:], h1P[:], ACT.Relu,
                                     bias=b1S[:, h:h + 1])
                h1.append(h1S)
            for g in range(GR // 128):
                oP = ps3.tile([128, cfg.LD], f32, tag="aggH")
                nc.tensor.matmul(oP[:], h1[0][:, g * 128:(g + 1) * 128],
                                 w2S[:, 0:cfg.LD], start=True, stop=False)
                nc.tensor.matmul(oP[:], h1[1][:, g * 128:(g + 1) * 128],
                                 w2S[:, cfg.LD:2 * cfg.LD], start=False, stop=True)
                oS = work.tile([128, cfg.LD], f32, tag="oS")
                nc.vector.tensor_tensor(oS[:], oP[:], b2B[:], AL.add)
                nc.sync.dma_start(out_ext[g * 128:(g + 1) * 128, :], oS[:])

    nc.compile()
    return nc


# ---------------------------------------------------------------------------
# Entry point
# ---------------------------------------------------------------------------
_CACHE = {}


def _build(cfg, meta):
    key = (tuple(meta.m_low), tuple(meta.m_high))
    if key not in _CACHE:
        _CACHE[key] = build_graph(cfg, meta)
    return _CACHE[key]


def kernel(**inputs) -> np.ndarray:
    from concourse.bass_utils import run_bass_kernel_spmd
    cfg = Cfg()
    meta = host_prep(cfg, **inputs)
    nc = _build(cfg, meta)
    res = run_bass_kernel_spmd(nc, meta.in_maps, list(range(cfg.C)))
    return np.asarray(res.results[0]["out"], dtype=np.float32)


# revision 20
# speedup vs baseline: 1.0443x; 1.0027x over previous
"""GCN (3-layer message passing + mean-pool + MLP head) on 8 Trainium2 NeuronCores.

v2.1: aggregate-then-matmul formulation.  Per layer the table holds
dinv-scaled features; each core dma_gathers its edges' src rows with 4-way
SWDGE queue rotation (descriptor generation parallelized over all four Q7
core pairs) and segment-sums them feature-major via indicator matmuls
(aggT = sum_k G_k-lhsT x Ind_k); the self-loop is one identity-matmul per
window against SBUF-resident local table rows, and the per-window W-matmul
runs post-aggregation (no transposes anywhere).  The GCN bias enters as a
rank-1 matmul into the same PSUM tile, so the whole epilogue runs on the
otherwise-idle Scalar engine.  Indicator one-hots are built once on DVE,
round-tripped through DRAM, and streamed back each layer.  Layer 1's table
is built locally (tiny degree AllGather + scale of the replicated input),
so only two 12.8MB feature AllGathers remain, each half-split and
overlapped with the opposite half's compute via an L/H two-pass window
loop.  Pooling is an indicator matmul + [129,256] AllReduce; the MLP head
is computed redundantly."""

import numpy as np
from dataclasses import dataclass, field


# ---------------------------------------------------------------------------
# Config
# ---------------------------------------------------------------------------
@dataclass
class Cfg:
    N: int = 50000          # nodes
    E: int = 600000         # edges
    F: int = 128            # feature dim
    NL: int = 3             # gcn layers
    G: int = 256            # graphs
    H: int = 256            # hidden dim of head
    LD: int = 2             # label dim
    C: int = 8              # cores
    GCHUNK: int = 1024      # edges per dma_gather call
    IB: int = 8             # indicator subchunks built per DVE op

    @property
    def NPC(self):          # nodes per core
        return self.N // self.C

    @property
    def W(self):            # 128-node windows per core
        return (self.NPC + 127) // 128

    @property
    def NPAD(self):         # padded nodes per core
        return self.W * 128

    @property
    def TROWS(self):        # gather-table rows
        return self.C * self.NPAD

    @property
    def HA(self):           # local rows in table half A (window-aligned)
        return 128 * ((self.W + 1) // 2)

    @property
    def HB(self):           # local rows in table half B
        return self.NPAD - self.HA

    @property
    def SPLIT(self):        # low/high gather-stream boundary = half-A rows
        return self.C * self.HA


@dataclass
class Meta:
    """Uniform (core-independent) graph structure + per-core data arrays."""
    m_low: list = field(default_factory=list)    # per-window low subchunk count
    m_high: list = field(default_factory=list)   # per-window high subchunk count
    L_low: int = 0
    L_high: int = 0
    in_maps: list = field(default_factory=list)  # per-core tensor dicts


# ---------------------------------------------------------------------------
# Host-side sharding / layout prep (pure numpy, no model math)
# ---------------------------------------------------------------------------
def _wrap16(arr_i16):
    # slot i -> [i % 16, i // 16]; 16-row wrap replicated to 128 partitions
    # (one copy per GPSIMD Q7 core).
    return np.ascontiguousarray(np.tile(arr_i16.reshape(-1, 16).T, (8, 1)))


def _wrap128(arr):
    # slot i -> [i % 128, i // 128]
    return np.ascontiguousarray(arr.reshape(-1, 128).T)


def host_prep(cfg: Cfg, x, Wg, bg, w1, b1, w2, b2, edge_index, batch) -> Meta:
    import ml_dtypes
    C, NPC, W, NPAD = cfg.C, cfg.NPC, cfg.W, cfg.NPAD
    src = np.asarray(edge_index[0], dtype=np.int64)
    dst = np.asarray(edge_index[1], dtype=np.int64)
    batch = np.asarray(batch, dtype=np.int64)
    x = np.asarray(x, dtype=np.float32)

    # table row of a global node id: rows [0, C*HA) hold every core's first
    # HA local rows, rows [C*HA, TROWS) the remaining HB.
    HA, HB = cfg.HA, cfg.HB
    nid = np.arange(cfg.N, dtype=np.int64)
    nc_, nl = nid // NPC, nid % NPC
    trow_of = np.where(nl < HA, nc_ * HA + nl, C * HA + nc_ * HB + (nl - HA))
    trow = trow_of[src]

    # replicated node features in table-row order (layer-1 table source)
    xfull = np.zeros((cfg.TROWS, cfg.F), dtype=ml_dtypes.bfloat16)
    xfull[trow_of] = x.astype(ml_dtypes.bfloat16)

    # per (core, window, half) edge lists
    per_core = []
    for c in range(C):
        m = (dst // NPC) == c
        s_c, d_c, t_c = src[m], dst[m], trow[m]
        dloc = d_c - c * NPC
        order = np.argsort(dloc, kind="stable")
        s_c, dloc, t_c = s_c[order], dloc[order], t_c[order]
        win = dloc // 128
        drel = dloc - win * 128
        lowm = t_c < cfg.SPLIT
        lists = []
        for w in range(W):
            wm = win == w
            lists.append((
                (t_c[wm & lowm], drel[wm & lowm]),
                (t_c[wm & ~lowm] - cfg.SPLIT, drel[wm & ~lowm]),
            ))
        per_core.append(lists)

    # uniform subchunk counts (max over cores), >=1 low subchunk per window
    m_low = [max(1, max(-(-len(per_core[c][w][0][0]) // 128) for c in range(C)))
             for w in range(W)]
    m_high = [max(-(-len(per_core[c][w][1][0]) // 128) for c in range(C))
              for w in range(W)]
    L_low = 128 * sum(m_low)
    L_high = 128 * sum(m_high)

    meta = Meta(m_low=m_low, m_high=m_high, L_low=L_low, L_high=L_high)

    for c in range(C):
        idx_low = np.zeros(L_low, np.int16)
        drel_low = np.full(L_low, -1.0, np.float32)
        idx_high = np.zeros(max(L_high, 128), np.int16)
        drel_high = np.full(max(L_high, 128), -1.0, np.float32)
        ol = oh = 0
        for w in range(W):
            (tl, dl), (th, dh) = per_core[c][w]
            idx_low[ol:ol + len(tl)] = tl.astype(np.int16)
            drel_low[ol:ol + len(dl)] = dl.astype(np.float32)
            ol += 128 * m_low[w]
            idx_high[oh:oh + len(th)] = th.astype(np.int16)
            drel_high[oh:oh + len(dh)] = dh.astype(np.float32)
            oh += 128 * m_high[w]
        assert ol == L_low and oh == L_high

        # local x shard, node-major bf16 (layer-1 self-loop table rows)
        xs = np.zeros((NPAD, cfg.F), ml_dtypes.bfloat16)
        xs[:NPC] = x[c * NPC:(c + 1) * NPC].astype(ml_dtypes.bfloat16)

        # pooling one-hot indicators [node-slot x G], bf16
        pind = np.zeros((NPAD, cfg.G), ml_dtypes.bfloat16)
        pind[np.arange(NPC), batch[c * NPC:(c + 1) * NPC]] = 1.0

        meta.in_maps.append(dict(
            xs=np.ascontiguousarray(xs),
            xfull=xfull,
            pind=np.ascontiguousarray(pind),
            src_low=_wrap16(idx_low),
            src_high=_wrap16(idx_high),
            drel_low=_wrap128(drel_low),
            drel_high=_wrap128(drel_high),
            Wg=np.asarray(Wg, np.float32),
            bg=np.asarray(bg, np.float32),
            w1=np.asarray(w1, np.float32),
            b1=np.asarray(b1, np.float32).reshape(cfg.H, 1),
            w2=np.asarray(w2, np.float32),
            b2=np.asarray(b2, np.float32).reshape(1, cfg.LD),
        ))
    return meta


# ---------------------------------------------------------------------------
# Device graph
# ---------------------------------------------------------------------------
def build_graph(cfg: Cfg, meta: Meta):
    import concourse.bass as bass
    import concourse.bacc as bacc
    import concourse.mybir as mybir
    import concourse.tile as tile

    F, W, NL, NPAD = cfg.F, cfg.W, cfg.NL, cfg.NPAD
    GR = cfg.G
    f32, bf16, i16 = mybir.dt.float32, mybir.dt.bfloat16, mybir.dt.int16
    AL = mybir.AluOpType
    ACT = mybir.ActivationFunctionType
    L_low, L_high = meta.L_low, meta.L_high
    LH_pad = max(L_high, 128)
    HA, WA = cfg.HA, cfg.HA // 128
    TROWS = cfg.TROWS
    WT = TROWS // 128
    WTA = cfg.C * HA // 128          # table windows in half A
    nsubL = L_low // 128
    nsubH = LH_pad // 128

    nc = bacc.Bacc("TRN2", target_bir_lowering=False, debug=False,
                   num_devices=cfg.C, num_swdge_queues=4)

    # --- external IO ------------------------------------------------------
    P = {}
    P["xs"] = nc.declare_dram_parameter("xs", [NPAD, F], bf16, isOutput=False)
    P["xfull"] = nc.declare_dram_parameter("xfull", [TROWS, F], bf16, isOutput=False)
    P["pind"] = nc.declare_dram_parameter("pind", [NPAD, GR], bf16, isOutput=False)
    P["src_low"] = nc.declare_dram_parameter("src_low", [128, L_low // 16], i16, isOutput=False)
    P["src_high"] = nc.declare_dram_parameter("src_high", [128, LH_pad // 16], i16, isOutput=False)
    P["drel_low"] = nc.declare_dram_parameter("drel_low", [128, nsubL], f32, isOutput=False)
    P["drel_high"] = nc.declare_dram_parameter("drel_high", [128, nsubH], f32, isOutput=False)
    P["Wg"] = nc.declare_dram_parameter("Wg", [NL, F, F], f32, isOutput=False)
    P["bg"] = nc.declare_dram_parameter("bg", [NL, F], f32, isOutput=False)
    P["w1"] = nc.declare_dram_parameter("w1", [F, cfg.H], f32, isOutput=False)
    P["b1"] = nc.declare_dram_parameter("b1", [cfg.H, 1], f32, isOutput=False)
    P["w2"] = nc.declare_dram_parameter("w2", [cfg.H, cfg.LD], f32, isOutput=False)
    P["b2"] = nc.declare_dram_parameter("b2", [1, cfg.LD], f32, isOutput=False)
    out_ext = nc.declare_dram_parameter("out", [GR, cfg.LD], f32, isOutput=True)

    # --- internal DRAM ----------------------------------------------------
    tableX = nc.dram_tensor("tableX", [TROWS, F], bf16)          # layer-1
    tableDs = [nc.dram_tensor(f"tableD{i}", [TROWS, F], bf16,
                              addr_space="Shared") for i in range(2)]
    shardDs = [nc.dram_tensor(f"shardD{i}", [NPAD, F], bf16) for i in range(2)]
    NSUB = nsubL + nsubH
    indD = nc.dram_tensor("indD", [128, NSUB * 128], bf16)       # one-hots
    degD = nc.dram_tensor("degD", [NPAD], f32)
    degAllD = nc.dram_tensor("degAllD", [TROWS], f32, addr_space="Shared")
    arInD = nc.dram_tensor("arInD", [129, GR], f32)
    arOutD = nc.dram_tensor("arOutD", [129, GR], f32, addr_space="Shared")

    rg = [list(range(cfg.C))]

    with tile.TileContext(nc) as tc:
        with (
            tc.tile_pool(name="res", bufs=1) as res,      # resident tensors
            tc.tile_pool(name="work", bufs=3) as work,    # per-window temps
            tc.tile_pool(name="indp", bufs=4) as indp,    # indicator batches
            tc.tile_pool(name="gbuf", bufs=3) as gpool,   # gather buffers
            tc.tile_pool(name="ps", bufs=2, space="PSUM") as ps,
            tc.tile_pool(name="ps3", bufs=2, space="PSUM") as ps3,
            tc.tile_pool(name="psacc", bufs=1, space="PSUM") as psacc,
        ):
            # ---------------- resident loads / constants ----------------
            srcL = res.tile([128, L_low // 16], i16)
            nc.sync.dma_start(srcL[:], P["src_low"][:])
            srcH = res.tile([128, LH_pad // 16], i16)
            nc.sync.dma_start(srcH[:], P["src_high"][:])
            drelL = res.tile([128, nsubL], f32)
            nc.sync.dma_start(drelL[:], P["drel_low"][:])
            drelH = res.tile([128, nsubH], f32)
            nc.sync.dma_start(drelH[:], P["drel_high"][:])

            iotaF = res.tile([128, 128], f32)   # value = free index
            nc.gpsimd.iota(iotaF[:], pattern=[[1, 128]], base=0,
                           channel_multiplier=0,
                           allow_small_or_imprecise_dtypes=True)
            iotaC = res.tile([128, 1], f32)    # value = partition index
            nc.gpsimd.iota(iotaC[:], pattern=[[0, 1]], base=0,
                           channel_multiplier=1,
                           allow_small_or_imprecise_dtypes=True)
            ident = res.tile([128, 128], bf16)  # identity (self-loop matmul)
            nc.vector.tensor_scalar(ident[:], iotaF[:], iotaC[:], None,
                                    AL.is_equal)
            onesB = res.tile([128, 1], bf16)
            nc.vector.memset(onesB[:], 1.0)
            onesRow = res.tile([1, 128], bf16)
            nc.vector.memset(onesRow[:], 1.0)

            IB = cfg.IB
            iotaT = res.tile([128, IB * 128], f32)
            nc.gpsimd.iota(iotaT[:], pattern=[[0, IB], [1, 128]], base=0,
                           channel_multiplier=0,
                           allow_small_or_imprecise_dtypes=True)

            # weights -> bf16 SBUF
            wg_f = work.tile([128, NL * F], f32, tag="wg_f")
            for l in range(NL):
                nc.sync.dma_start(wg_f[:, l * F:(l + 1) * F], P["Wg"][l])
            wgS = res.tile([128, NL * F], bf16)
            nc.vector.tensor_copy(wgS[:], wg_f[:])

            bg_row = res.tile([1, NL * F], f32)
            nc.sync.dma_start(bg_row[:], P["bg"][:].rearrange("l f -> (l f)"))
            bgRowB = res.tile([1, NL * F], bf16)   # rank-1 bias matmul rhs
            nc.vector.tensor_copy(bgRowB[:], bg_row[:])

            w1S = res.tile([128, cfg.H], f32)
            nc.sync.dma_start(w1S[:], P["w1"][:])
            w2S = res.tile([128, 2 * cfg.LD], f32)
            nc.sync.dma_start(w2S[:, 0:cfg.LD], P["w2"][0:128, :])
            nc.sync.dma_start(w2S[:, cfg.LD:2 * cfg.LD], P["w2"][128:256, :])

            b1S = res.tile([128, 2], f32)
            nc.sync.dma_start(b1S[:, 0:1], P["b1"][0:128, :])
            nc.sync.dma_start(b1S[:, 1:2], P["b1"][128:256, :])
            b2_row = res.tile([1, cfg.LD], f32)
            nc.sync.dma_start(b2_row[:], P["b2"][:])
            b2B = res.tile([128, cfg.LD], f32)
            nc.gpsimd.partition_broadcast(b2B[:], b2_row[:])

            xsB = res.tile([128, W * F], bf16)   # local x, node-major
            nc.sync.dma_start(
                xsB[:].rearrange("p (w f) -> p w f", f=F),
                P["xs"][:].rearrange("(w p) f -> p w f", p=128))

            Tbuf = res.tile([128, W * F], bf16)  # dinv*x_l local rows
            SL = res.tile([128, W * F], f32)     # pass-L partial aggT
            degRow = res.tile([1, W * 128], f32)
            dinvS = res.tile([128, W], f32)
            dinvA = res.tile([128, WT], f32)     # table-row dinv (all cores)

            # per-window subchunk schedule, split by stream
            schedL, schedH = [], []
            colL = colH = 0
            for w in range(W):
                lst = []
                for _ in range(meta.m_low[w]):
                    lst.append(("L", colL))
                    colL += 1
                schedL.append(lst)
                lst = []
                for _ in range(meta.m_high[w]):
                    lst.append(("H", colH))
                    colH += 1
                schedH.append(lst)

            def flat_col(stream, col):
                return col if stream == "L" else nsubL + col

            def make_ind_builder():
                """JIT builder (deg pass): builds batches on DVE and writes
                each batch to indD for later streaming."""
                cache = {}

                def get(stream, col):
                    drel, nsub = (drelL, nsubL) if stream == "L" else (drelH, nsubH)
                    s0 = col - col % IB
                    key = (stream, s0)
                    if key not in cache:
                        nb = min(IB, nsub - s0)
                        it = indp.tile([128, IB * 128], bf16, tag="ind")
                        dsl = drel[:, s0:s0 + nb]
                        din = bass.AP(dsl.tensor, dsl.offset,
                                      [list(d) for d in dsl.ap] + [[0, 128]])
                        nc.vector.tensor_tensor(
                            it[:].rearrange("p (c e) -> p c e", e=128)[:, 0:nb, :],
                            din,
                            iotaT[:].rearrange("p (c e) -> p c e", e=128)[:, 0:nb, :],
                            AL.is_equal)
                        fc = flat_col(stream, s0)
                        nc.sync.dma_start(
                            indD[:, fc * 128:(fc + nb) * 128],
                            it[:, 0:nb * 128])
                        cache[key] = it
                        for k in [k for k in cache
                                  if k[0] == stream and k[1] < s0 - IB]:
                            del cache[k]
                    return (cache[key][:]
                            .rearrange("p (c e) -> p c e", e=128)[:, col - s0, :])
                return get

            SB = 16   # chunks per streamed indicator batch

            def make_ind_streamer():
                """Per-layer getter: streams prebuilt one-hots from indD
                (issued on the ACT engine to keep the SP queue clear)."""
                cache = {}

                def get(stream, col):
                    nsub = nsubL if stream == "L" else nsubH
                    s0 = col - col % SB
                    key = (stream, s0)
                    if key not in cache:
                        nb = min(SB, nsub - s0)
                        it = indp.tile([128, SB * 128], bf16, tag="inds")
                        fc = flat_col(stream, s0)
                        nc.scalar.dma_start(
                            it[:, 0:nb * 128],
                            indD[:, fc * 128:(fc + nb) * 128])
                        cache[key] = it
                        for k in [k for k in cache
                                  if k[0] == stream and k[1] < s0 - SB]:
                            del cache[k]
                    return (cache[key][:]
                            .rearrange("p (c e) -> p c e", e=128)[:, col - s0, :])
                return get

            # gather-call layout per stream
            def gather_calls(L_tot):
                calls = []
                s = 0
                while s < L_tot:
                    n = min(cfg.GCHUNK, L_tot - s)
                    calls.append((s, n))
                    s += n
                return calls
            callsL = gather_calls(L_low)
            callsH = gather_calls(L_high)

            qrot = [0]

            def emit_calls(l, gtiles, stream):
                tbl = tableX if l == 0 else tableDs[l % 2]
                srcT = srcL if stream == "L" else srcH
                r0, r1 = ((0, cfg.SPLIT) if stream == "L"
                          else (cfg.SPLIT, TROWS))
                for (s0, n) in (callsL if stream == "L" else callsH):
                    q = qrot[0] % 4
                    qrot[0] += 1
                    gt = gpool.tile([128, (n // 128) * F], bf16, tag=f"g{q}")
                    nc.gpsimd.dma_gather(
                        gt[:].rearrange("p (c e) -> p c e", e=F),
                        tbl[r0:r1, :],
                        srcT[:, s0 // 16:(s0 + n) // 16],
                        n, n, F, queue_num=q)
                    gtiles[(stream, s0)] = gt

            def gslice(gtiles, stream, col):
                calls = callsL if stream == "L" else callsH
                for (s0, n) in calls:
                    if s0 <= col * 128 < s0 + n:
                        gt = gtiles[(stream, s0)]
                        j = col - s0 // 128
                        return gt[:].rearrange("p (c e) -> p c e", e=F)[:, j, :]
                raise AssertionError

            # ---------------- degree pass + layer-1 table ----------------
            # per half: deg windows -> deg AG -> dinv -> scaled table build
            get_ind = make_ind_builder()
            for w in range(W):
                degP = ps3.tile([1, 128], f32, tag="aggL")
                sched_w = schedL[w] + schedH[w]
                n = len(sched_w)
                for i, (stream, col) in enumerate(sched_w):
                    nc.tensor.matmul(degP[:], onesB[:], get_ind(stream, col),
                                     start=(i == 0), stop=(i == n - 1))
                nc.scalar.activation(degRow[:, w * 128:(w + 1) * 128],
                                     degP[:], ACT.Copy)
                if w == WA - 1:
                    nc.sync.dma_start(
                        degD[0:HA].rearrange("(a b) -> a b", a=1),
                        degRow[:, 0:HA])
                    nc.gpsimd.collective_compute(
                        "AllGather", mybir.AluOpType.bypass, replica_groups=rg,
                        ins=[degD[0:HA]], outs=[degAllD[0:cfg.C * HA]])
            nc.sync.dma_start(
                degD[HA:NPAD].rearrange("(a b) -> a b", a=1),
                degRow[:, HA:NPAD])
            nc.gpsimd.collective_compute(
                "AllGather", mybir.AluOpType.bypass, replica_groups=rg,
                ins=[degD[HA:NPAD]], outs=[degAllD[cfg.C * HA:TROWS]])

            # local dinv (epilogue scaling)
            degCol = res.tile([128, W], f32)
            nc.sync.dma_start(
                degCol[:], degD[:].rearrange("(w p) -> p w", p=128))
            sq = work.tile([128, W], f32, tag="sq")
            nc.scalar.activation(sq[:], degCol[:], ACT.Sqrt, bias=1.0)
            nc.vector.reciprocal(dinvS[:], sq[:])

            # per-half: table-row dinv + scaled layer-1 table build
            TB = 8   # table windows per build chunk
            for (j0, j1) in ((0, WTA), (WTA, WT)):
                degAllS = work.tile([128, WT], f32, tag="degAllS")
                nc.sync.dma_start(
                    degAllS[:, j0:j1],
                    degAllD[j0 * 128:j1 * 128].rearrange("(w p) -> p w", p=128))
                sqA = work.tile([128, WT], f32, tag="sqA")
                nc.scalar.activation(sqA[:, j0:j1], degAllS[:, j0:j1],
                                     ACT.Sqrt, bias=1.0)
                nc.vector.reciprocal(dinvA[:, j0:j1], sqA[:, j0:j1])
                for t0 in range(j0, j1, TB):
                    t1 = min(t0 + TB, j1)
                    xbt = work.tile([128, TB * F], bf16, tag="xbt")
                    nc.sync.dma_start(
                        xbt[:, 0:(t1 - t0) * F].rearrange("p (w f) -> p w f", f=F),
                        P["xfull"][t0 * 128:t1 * 128, :]
                        .rearrange("(w p) f -> p w f", p=128))
                    tbt = work.tile([128, TB * F], bf16, tag="tbt")
                    for t in range(t0, t1):
                        if t % 3 == 2:
                            # every third window scaled on DVE (broadcast AP)
                            # to parallelize the build with the ACT engine
                            dsl = dinvA[:, t:t + 1]
                            dbc = bass.AP(dsl.tensor, dsl.offset,
                                          [list(dsl.ap[0]), [0, F]])
                            nc.vector.tensor_tensor(
                                tbt[:, (t - t0) * F:(t - t0 + 1) * F],
                                xbt[:, (t - t0) * F:(t - t0 + 1) * F],
                                dbc, AL.mult)
                        else:
                            nc.scalar.activation(
                                tbt[:, (t - t0) * F:(t - t0 + 1) * F],
                                xbt[:, (t - t0) * F:(t - t0 + 1) * F],
                                ACT.Copy, scale=dinvA[:, t:t + 1])
                    nc.sync.dma_start(
                        tableX[t0 * 128:t1 * 128, :]
                        .rearrange("(w p) f -> p w f", p=128),
                        tbt[:, 0:(t1 - t0) * F].rearrange("p (w f) -> p w f", f=F))

            # ---------------- layers -------------------------------------
            poolP = None
            cntP = None
            for l in range(NL):
                gtiles = {}
                get_ind = make_ind_streamer()
                last = l == NL - 1

                emit_calls(l, gtiles, "L")
                if l == 0:
                    emit_calls(l, gtiles, "H")

                # ---- pass L: low-stream partial aggregates -> SL ----
                for w in range(W):
                    aggLP = ps3.tile([128, F], f32, tag="aggL")
                    n = len(schedL[w])
                    for i, (stream, col) in enumerate(schedL[w]):
                        nc.tensor.matmul(aggLP[:], gslice(gtiles, stream, col),
                                         get_ind(stream, col),
                                         start=(i == 0), stop=(i == n - 1))
                    nc.scalar.activation(SL[:, w * F:(w + 1) * F], aggLP[:],
                                         ACT.Copy)

                if l > 0:
                    emit_calls(l, gtiles, "H")

                if last:
                    poolP = psacc.tile([128, GR], f32, tag="poolP")
                    cntP = psacc.tile([1, GR], f32, tag="acc1")

                # ---- pass H: high stream + self loop + epilogue ----
                for w in range(W):
                    if l == 0:
                        nc.scalar.activation(
                            Tbuf[:, w * F:(w + 1) * F],
                            xsB[:, w * F:(w + 1) * F],
                            ACT.Copy, scale=dinvS[:, w:w + 1])
                    aggP = ps3.tile([128, F], f32, tag="aggH")
                    for i, (stream, col) in enumerate(schedH[w]):
                        nc.tensor.matmul(aggP[:], gslice(gtiles, stream, col),
                                         get_ind(stream, col),
                                         start=(i == 0), stop=False)
                    nc.tensor.matmul(aggP[:], Tbuf[:, w * F:(w + 1) * F],
                                     ident[:],
                                     start=(len(schedH[w]) == 0), stop=True)
                    aT = work.tile([128, F], bf16, tag="aT")
                    nc.vector.tensor_tensor(aT[:], aggP[:],
                                            SL[:, w * F:(w + 1) * F], AL.add)
                    xP = ps.tile([128, F], f32, tag="xP")
                    nc.tensor.matmul(xP[:], aT[:], wgS[:, l * F:(l + 1) * F],
                                     start=True, stop=False)
                    nc.tensor.matmul(xP[:], onesRow[:],
                                     bgRowB[:, l * F:(l + 1) * F],
                                     start=False, stop=True)
                    xn = work.tile([128, F], bf16, tag="xn")
                    nc.scalar.activation(xn[:], xP[:], ACT.Relu,
                                         scale=dinvS[:, w:w + 1])

                    if not last:
                        nc.scalar.activation(
                            Tbuf[:, w * F:(w + 1) * F], xn[:],
                            ACT.Copy, scale=dinvS[:, w:w + 1])
                        if w == WA - 1:
                            shardD = shardDs[(l + 1) % 2]
                            tableD = tableDs[(l + 1) % 2]
                            nc.sync.dma_start(
                                shardD[0:HA].rearrange("(w p) f -> p w f", p=128),
                                Tbuf[:, 0:HA * F // 128]
                                .rearrange("p (w f) -> p w f", f=F))
                            nc.gpsimd.collective_compute(
                                "AllGather", mybir.AluOpType.bypass,
                                replica_groups=rg,
                                ins=[shardD[0:HA]],
                                outs=[tableD[0:cfg.C * HA]])
                    else:
                        pw = work.tile([128, GR], bf16, tag="pw")
                        nc.sync.dma_start(
                            pw[:], P["pind"][w * 128:(w + 1) * 128, :])
                        nc.tensor.matmul(
                            poolP[:], xn[:], pw[:],
                            start=(w == 0), stop=(w == W - 1),
                            skip_group_check=True)
                        nc.tensor.matmul(
                            cntP[:], onesB[:], pw[:],
                            start=(w == 0), stop=(w == W - 1),
                            skip_group_check=True)

                if not last:
                    shardD = shardDs[(l + 1) % 2]
                    tableD = tableDs[(l + 1) % 2]
                    nc.sync.dma_start(
                        shardD[HA:NPAD].rearrange("(w p) f -> p w f", p=128),
                        Tbuf[:, HA * F // 128:NPAD * F // 128]
                        .rearrange("p (w f) -> p w f", f=F))
                    nc.gpsimd.collective_compute(
                        "AllGather", mybir.AluOpType.bypass, replica_groups=rg,
                        ins=[shardD[HA:NPAD]],
                        outs=[tableD[cfg.C * HA:TROWS]])

            # ---------------- pooling allreduce + head ----------------
            sumsS = work.tile([128, GR], f32, tag="sumsS")
            nc.vector.tensor_copy(sumsS[:], poolP[:])
            cntS = work.tile([1, GR], f32, tag="cntS")
            nc.vector.tensor_copy(cntS[:], cntP[:])
            nc.sync.dma_start(arInD[0:128, :], sumsS[:])
            nc.sync.dma_start(arInD[128:129, :], cntS[:])
            nc.gpsimd.collective_compute(
                "AllReduce", mybir.AluOpType.add, replica_groups=rg,
                ins=[arInD[:]], outs=[arOutD[:]])
            sumsA = work.tile([128, GR], f32, tag="sumsA")
            nc.sync.dma_start(sumsA[:], arOutD[0:128, :])
            cntA = work.tile([1, GR], f32, tag="cntA")
            nc.sync.dma_start(cntA[:], arOutD[128:129, :])
            cntM = work.tile([1, GR], f32, tag="cntM")
            nc.vector.tensor_scalar(cntM[:], cntA[:], 1.0, None, AL.max)
            rec = work.tile([1, GR], f32, tag="rec")
            nc.vector.reciprocal(rec[:], cntM[:])
            recB = work.tile([128, GR], f32, tag="recB")
            nc.gpsimd.partition_broadcast(recB[:], rec[:])
            pooledT = work.tile([128, GR], f32, tag="pooledT")
            nc.vector.tensor_tensor(pooledT[:], sumsA[:], recB[:], AL.mult)

            h1 = []
            for h in range(2):
                h1P = ps3.tile([128, GR], f32, tag="aggL")
                nc.tensor.matmul(h1P[:], w1S[:, h * 128:(h + 1) * 128],
                                 pooledT[:], start=True, stop=True)
                h1S = work.tile([128, GR], f32, tag=f"h1S{h}")
                nc.scalar.activation(h1S[:], h1P[:], ACT.Relu,
                                     bias=b1S[:, h:h + 1])
                h1.append(h1S)
            for g in range(GR // 128):
                oP = ps3.tile([128, cfg.LD], f32, tag="aggH")
                nc.tensor.matmul(oP[:], h1[0][:, g * 128:(g + 1) * 128],
                                 w2S[:, 0:cfg.LD], start=True, stop=False)
                nc.tensor.matmul(oP[:], h1[1][:, g * 128:(g + 1) * 128],
                                 w2S[:, cfg.LD:2 * cfg.LD], start=False, stop=True)
                oS = work.tile([128, cfg.LD], f32, tag="oS")
                nc.vector.tensor_tensor(oS[:], oP[:], b2B[:], AL.add)
                nc.sync.dma_start(out_ext[g * 128:(g + 1) * 128, :], oS[:])

    nc.compile()
    return nc


# ---------------------------------------------------------------------------
# Entry point
# ---------------------------------------------------------------------------
_CACHE = {}


def _build(cfg, meta):
    key = (tuple(meta.m_low), tuple(meta.m_high))
    if key not in _CACHE:
        _CACHE[key] = build_graph(cfg, meta)
    return _CACHE[key]


def kernel(**inputs) -> np.ndarray:
    from concourse.bass_utils import run_bass_kernel_spmd
    cfg = Cfg()
    meta = host_prep(cfg, **inputs)
    nc = _build(cfg, meta)
    res = run_bass_kernel_spmd(nc, meta.in_maps, list(range(cfg.C)))
    return np.asarray(res.results[0]["out"], dtype=np.float32)
